# revision 3
# baseline (speedup 1.0000x reference)
"""KNRM kernel v3 for 8 TRN2 NeuronCores (data-parallel over batch).

Structure per core (512 batch rows, 16 sub-batches of 32 rows):
- Phase A: 5 big indirect gathers (2048 rows of 128B each) pull all
  10240 token embeddings (fp16, padded to 64 cols) for a sub-batch into
  SBUF in slot order: d-slots [0,8192) = unit u*128+dpos, q-slots
  [8192,10240) = 8192 + u*32 + qpos (u = pair*32 + row, 2 pad qpos).
- PE transposes [128,50] -> PSUM [50,2048] (16 ranks/group), then 5 wide
  ACT/DVE copies cast f32->fp16 into T [64,10240] (emb-dim on partitions).
- 64 matmuls per sub-batch: mm[q,d] cosine blocks into PSUM [128,2048]
  (4 row-quadrants x 16 col-blocks of 128 d-tokens).
- RBF: V=exp(10x); per group of 5 kernels: Square(bias=-mu), exp(-50 sq),
  chain mults by V (bf16), 21 reduces to S (f32). Exact-match kernel via
  V >= 1e4 count.
- Tail: st=S*dconst, log1p, *w, reduce -> r2 into R2ALL; once at core
  end: 2 matmuls vs q4 selector -> logits [128,4]x2, subtract, sigmoid,
  one DMA out.
"""
import math
import numpy as np

import concourse.bass as bass
import concourse.bacc as bacc
import concourse.mybir as mybir
import concourse.tile as tile
from concourse import bass_utils

F32 = mybir.dt.float32
BF16 = mybir.dt.bfloat16
FP16 = mybir.dt.float16
I32 = mybir.dt.int32
AF = mybir.ActivationFunctionType
ALU = mybir.AluOpType

N_CORES = 8
B, LQ, LD = 4096, 30, 128
V, D = 400000, 50
DP = 64                    # f32 row padded to 64 elems = 256B
KN = 21
BC = B // N_CORES          # 512 rows per core
SB = 16                    # sub-batches per core
RS = 32                    # rows per sub-batch
NU = 64                    # units (row,pair) per sub-batch
DSLOTS = NU * 128          # 8192
SLOTS = DSLOTS + NU * 32   # 10240
RANKS = SLOTS // 128       # 80

MUS = []
for i in range(KN):
    mu = 1.0 / (KN - 1) + 2.0 * i / (KN - 1) - 1.0
    MUS.append(min(mu, 1.0))
GROUPS = [(k0, MUS[k0]) for k0 in (0, 5, 10, 15)]
DCONST = np.zeros(KN, np.float64)
for k0, mu_f in GROUPS:
    for j in range(5):
        DCONST[k0 + j] = math.exp(-10.0 * j * mu_f - j * j / 2.0)
DCONST[20] = 1.0


def _build_nc():
    nc = bacc.Bacc("TRN2", target_bir_lowering=False, debug=False,
                   num_devices=N_CORES)
    embp = nc.dram_tensor("embp", [V, DP], FP16, kind="ExternalInput")
    aidx = nc.dram_tensor("aidx", [128, SB * RANKS], I32, kind="ExternalInput")
    ident = nc.dram_tensor("ident", [128, 128], FP16, kind="ExternalInput")
    q4 = nc.dram_tensor("q4", [128, 4], F32, kind="ExternalInput")
    dwf = nc.dram_tensor("dwf", [128, 16 * KN], F32, kind="ExternalInput")
    wtf = nc.dram_tensor("wtf", [128, 16 * KN], F32, kind="ExternalInput")
    y = nc.dram_tensor("y", [BC, 1], F32, kind="ExternalOutput")

    with tile.TileContext(nc) as tc, nc.allow_low_precision("bf16 rbf chain"):
        with (
            tc.tile_pool(name="const", bufs=1) as cpool,
            tc.tile_pool(name="abuf", bufs=2) as apool,
            tc.tile_pool(name="tbuf", bufs=2) as tpool,
            tc.tile_pool(name="v", bufs=2) as vpool,
            tc.tile_pool(name="sq", bufs=2) as sqpool,
            tc.tile_pool(name="ch", bufs=3) as chpool,
            tc.tile_pool(name="s", bufs=2) as spool,
            tc.tile_pool(name="tail", bufs=2) as tlpool,
            tc.tile_pool(name="r2", bufs=1) as r2pool,
            tc.tile_pool(name="fin", bufs=1) as fpool,
            tc.tile_pool(name="mm", bufs=1, space="PSUM") as mmpool,
        ):
            identt = cpool.tile([128, 128], FP16, tag="ident")
            nc.sync.dma_start(out=identt[:], in_=ident.ap())
            q4t = cpool.tile([128, 4], F32, tag="q4")
            nc.sync.dma_start(out=q4t[:], in_=q4.ap())
            dwt = cpool.tile([128, 16 * KN], F32, tag="dw")
            nc.sync.dma_start(out=dwt[:], in_=dwf.ap())
            wtt = cpool.tile([128, 16 * KN], F32, tag="wt")
            nc.sync.dma_start(out=wtt[:], in_=wtf.ap())
            aidxt = cpool.tile([128, SB * RANKS], I32, tag="aidx")
            nc.sync.dma_start(out=aidxt[:], in_=aidx.ap())
            bias_aps = {}
            for (k0g, mu_g) in GROUPS:
                bt = cpool.tile([128, 1], F32, tag=f"bias{k0g}",
                                name=f"bias{k0g}")
                nc.vector.memset(bt[:], -float(mu_g))
                bias_aps[k0g] = bt
            r2all = r2pool.tile([128, 2, 128], F32, tag="r2all")

            with tc.tile_pool(name="pt", bufs=2, space="PSUM") as ptpool:
                for sb in range(SB):
                    # ---- phase A: 80 indirect gathers (128 rows each) ----
                    at2 = apool.tile([128, RANKS * DP], FP16, tag="A")
                    at = at2[:].rearrange("p (a b) -> p a b", b=DP)
                    for t in range(RANKS):
                        nc.gpsimd.indirect_dma_start(
                            out=at2[:, t * DP:(t + 1) * DP],
                            out_offset=None, in_=embp.ap(),
                            in_offset=bass.IndirectOffsetOnAxis(
                                ap=aidxt[:, sb * RANKS + t:
                                         sb * RANKS + t + 1],
                                axis=0))
                    # ---- transpose to [emb-dim, slot] ----
                    tt = tpool.tile([64, SLOTS], FP16, tag="T")
                    for blk in range(5):
                        pt = ptpool.tile([64, 2048], FP16, tag="pt")
                        for t2 in range(16):
                            t = blk * 16 + t2
                            nc.tensor.transpose(
                                out=pt[0:50, t2 * 128:(t2 + 1) * 128],
                                in_=at[:, t, 0:50], identity=identt[:])
                        dst = tt[0:50, blk * 2048:(blk + 1) * 2048]
                        if blk % 2 == 0:
                            nc.scalar.activation(dst, pt[0:50, :], AF.Copy)
                        else:
                            nc.vector.tensor_copy(out=dst, in_=pt[0:50, :])
                    # ---- matmuls ----
                    mmt = mmpool.tile([128, 2048], F32, tag="mm")
                    for u in range(NU):
                        qd, i = u % 4, u // 4
                        nc.tensor.matmul(
                            out=mmt[32 * qd:32 * (qd + 1),
                                    128 * i:128 * (i + 1)],
                            lhsT=tt[0:50, DSLOTS + 32 * u:DSLOTS + 32 * (u + 1)],
                            rhs=tt[0:50, 128 * u:128 * (u + 1)],
                            start=True, stop=True,
                            tile_position=(0, 32 * qd))
                    # ---- RBF ----
                    vt = vpool.tile([128, 2048], BF16, tag="V")
                    nc.scalar.activation(vt[:], mmt[:], AF.Exp, scale=10.0)
                    st_ = spool.tile([128, 16, KN], F32, tag="S")
                    for (k0, mu_f) in GROUPS:
                        sqt = sqpool.tile([128, 2048], FP16, tag="sq")
                        nc.scalar.activation(sqt[:], mmt[:], AF.Square,
                                             bias=bias_aps[k0][:])
                        kt = chpool.tile([128, 2048], BF16, tag=f"K{k0 % 3}")
                        nc.scalar.activation(kt[:], sqt[:], AF.Exp,
                                             scale=-50.0)
                        nc.vector.tensor_reduce(
                            out=st_[:, :, k0:k0 + 1],
                            in_=kt[:].rearrange("p (t r) -> p t r", r=128),
                            axis=mybir.AxisListType.X, op=ALU.add)
                        for j in range(1, 5):
                            kt2 = chpool.tile([128, 2048], BF16,
                                              tag=f"K{(k0 + j) % 3}")
                            nc.vector.tensor_tensor(out=kt2[:], in0=kt[:],
                                                    in1=vt[:], op=ALU.mult)
                            nc.vector.tensor_reduce(
                                out=st_[:, :, k0 + j:k0 + j + 1],
                                in_=kt2[:].rearrange("p (t r) -> p t r",
                                                     r=128),
                                axis=mybir.AxisListType.X, op=ALU.add)
                            kt = kt2
                    ind = chpool.tile([128, 2048], BF16, tag="K0")
                    nc.vector.tensor_scalar(out=ind[:], in0=vt[:],
                                            scalar1=10000.0, scalar2=None,
                                            op0=ALU.is_ge)
                    nc.vector.tensor_reduce(
                        out=st_[:, :, 20:21],
                        in_=ind[:].rearrange("p (t r) -> p t r", r=128),
                        axis=mybir.AxisListType.X, op=ALU.add)
                    # ---- per-sub-batch tail ----
                    stt = tlpool.tile([128, 16 * KN], F32, tag="St")
                    nc.vector.tensor_tensor(
                        out=stt[:], in0=st_[:].rearrange("p t k -> p (t k)"),
                        in1=dwt[:], op=ALU.mult)
                    lt = tlpool.tile([128, 16 * KN], F32, tag="Lt")
                    nc.scalar.activation(lt[:], stt[:], AF.Ln, bias=1.0)
                    lw = tlpool.tile([128, 16 * KN], F32, tag="Lw")
                    nc.vector.tensor_tensor(out=lw[:], in0=lt[:],
                                            in1=wtt[:], op=ALU.mult)
                    lw3 = lw[:].rearrange("p (t k) -> p t k", k=KN)
                    nc.vector.tensor_reduce(
                        out=r2all[:, 0, sb * 8:sb * 8 + 8],
                        in_=lw3[:, 0:8, :],
                        axis=mybir.AxisListType.X, op=ALU.add)
                    nc.vector.tensor_reduce(
                        out=r2all[:, 1, sb * 8:sb * 8 + 8],
                        in_=lw3[:, 8:16, :],
                        axis=mybir.AxisListType.X, op=ALU.add)

            # ---- final tail (ptpool closed; 1 PSUM bank free) ----
            with tc.tile_pool(name="lg", bufs=1, space="PSUM") as lgpool:
                lg = lgpool.tile([128, 8], F32, tag="lg")
                nc.tensor.matmul(out=lg[:, 0:4], lhsT=r2all[:, 0, :],
                                 rhs=q4t[:], start=True, stop=True)
                nc.tensor.matmul(out=lg[:, 4:8], lhsT=r2all[:, 1, :],
                                 rhs=q4t[:], start=True, stop=True)
                lgs = fpool.tile([128, 8], F32, tag="lgs")
                nc.vector.tensor_copy(out=lgs[:], in_=lg[:])
                dif = fpool.tile([128, 4], F32, tag="dif")
                nc.vector.tensor_tensor(out=dif[:], in0=lgs[:, 0:4],
                                        in1=lgs[:, 4:8], op=ALU.subtract)
                sg = fpool.tile([128, 4], F32, tag="sg")
                nc.scalar.activation(sg[:], dif[:], AF.Sigmoid)
                nc.sync.dma_start(
                    out=y.ap().rearrange("(s i q) o -> (s i) (q o)",
                                         i=8, q=4),
                    in_=sg[:])
    nc.compile()
    return nc


_NC_CACHE = None


def _get_nc():
    global _NC_CACHE
    if _NC_CACHE is None:
        _NC_CACHE = _build_nc()
    return _NC_CACHE


def _host_prep(q1, d1, q2, d2, emb, mlp_w):
    q1 = np.asarray(q1); d1 = np.asarray(d1)
    q2 = np.asarray(q2); d2 = np.asarray(d2)
    emb = np.asarray(emb, dtype=np.float64)
    w = np.asarray(mlp_w, dtype=np.float64).reshape(-1)

    nrm = np.sqrt((emb ** 2).sum(axis=1, keepdims=True))
    embn = emb / np.maximum(nrm, 1e-12)
    embp = np.zeros((V, DP), np.float16)
    embp[:, 0:D] = embn.astype(np.float16)

    ident = np.eye(128, dtype=np.float16)
    q4 = np.zeros((128, 4), np.float32)
    for j in range(4):
        q4[32 * j:32 * j + 30, j] = 1.0
    dwf = np.broadcast_to(np.tile(DCONST.astype(np.float32), 16),
                          (128, 16 * KN)).copy()
    wtf = np.broadcast_to(np.tile(w.astype(np.float32), 16),
                          (128, 16 * KN)).copy()

    in_maps = []
    for c in range(N_CORES):
        b0 = c * BC
        aid = np.empty((128, SB * RANKS), np.int32)
        for sb in range(SB):
            r0 = b0 + sb * RS
            dd = np.concatenate([d1[r0:r0 + RS], d2[r0:r0 + RS]], axis=0)
            aid[:, sb * RANKS:sb * RANKS + NU] = dd.T
            qq = np.zeros((NU, 32), np.int32)
            qq[0:RS, 0:30] = q1[r0:r0 + RS]
            qq[RS:NU, 0:30] = q2[r0:r0 + RS]
            aid[:, sb * RANKS + NU:sb * RANKS + RANKS] = \
                qq.reshape(16, 128).T
        in_maps.append({"embp": embp, "aidx": aid, "ident": ident,
                        "q4": q4, "dwf": dwf, "wtf": wtf})
    return in_maps


def kernel(q1, d1, q2, d2, emb, mlp_w, mlp_b):
    in_maps = _host_prep(q1, d1, q2, d2, emb, mlp_w)
    nc = _get_nc()
    res = bass_utils.run_bass_kernel_spmd(nc, in_maps,
                                          core_ids=list(range(N_CORES)))
    out = np.concatenate([res.results[c]["y"] for c in range(N_CORES)],
                         axis=0)
    return out.astype(np.float32)
